# revision 6
# baseline (speedup 1.0000x reference)
"""DeepQI dense MLP on 8 Trainium2 NeuronCores.

Strategy (data-parallel, feature-major):
  - Shard batch B=16384 across 8 cores (2048 rows each); weights replicated.
  - On-chip layout is feature-major: activations live as [128, KT, B_c]
    SBUF tiles (feature on partitions, batch on free dim), so every layer is
    a matmul with K on partitions, zero transposes between layers, and
    BatchNorm stats are free-dim reductions.
  - Pairwise features qi.T = (x_i * x_j)/255 are built on-chip: A = PII @ xT,
    Bm = PJJ @ xT via 0/1 selection matmuls (K=64), then one DVE
    scalar_tensor_tensor pass (A * (1/255)) * Bm -> bf16.
  - BatchNorm is sync-BN: per-core bn_stats/bn_aggr (DVE), tiny [128, MT, 2]
    AllReduce of (mean/8, E[x^2]/8) per layer, then scale/shift+relu fused in
    one ACT pass per m-tile.  Linear biases b0/b1/b2 cancel in BN and are
    dropped entirely.
  - All matmuls in bf16 (fp32 PSUM accumulate): measured end-to-end rel err
    ~7e-3 vs the fp32 reference.
"""

import numpy as np
import ml_dtypes

import concourse.bass as bass
import concourse.mybir as mybir
import concourse.tile as tile
from concourse import bacc
from concourse.bass import ts
from concourse.bass_utils import run_bass_kernel_spmd

N_CORES = 8
P = 128
B = 16384
BC = B // N_CORES  # 2048 batch rows per core
F = 256            # xv feature dim
FIELD = 64
NPAIR = 2016
NPAIR_PAD = 2048   # pad pairs to 16 full k-tiles
D1, D2, D3 = 2048, 2048, 1024
NCHUNK = 512       # matmul moving free dim (one PSUM bank)
EPS = 1e-5

BF16 = mybir.dt.bfloat16
F32 = mybir.dt.float32
AF = mybir.ActivationFunctionType
ALU = mybir.AluOpType

_II, _JJ = np.triu_indices(FIELD, k=1)

KT0 = (F + NPAIR_PAD) // P          # 18
MT1, MT2, MT3 = D1 // P, D2 // P, D3 // P   # 16, 16, 8
KT3 = D3 // P                        # 8


def build_nc(bc=BC, mm_reps=1, net_reps=1):
    # mm_reps > 1 redundantly recomputes every m-strip; net_reps > 1
    # redundantly re-runs the ENTIRE forward pass (identical results) —
    # used only to amplify device time above the axon relay's wall-clock
    # noise floor for timing.
    assert bc % NCHUNK == 0
    nch = bc // NCHUNK

    nc = bacc.Bacc("TRN2", target_bir_lowering=False, debug=False,
                   num_devices=N_CORES)

    xt_d = nc.dram_tensor("xt", [2, P, bc], BF16, kind="ExternalInput")
    pii_d = nc.dram_tensor("pii", [FIELD, NPAIR_PAD], BF16, kind="ExternalInput")
    pjj_d = nc.dram_tensor("pjj", [FIELD, NPAIR_PAD], BF16, kind="ExternalInput")
    w0_d = nc.dram_tensor("w0t", [KT0, P, D1], BF16, kind="ExternalInput")
    w1_d = nc.dram_tensor("w1t", [MT1, P, D2], BF16, kind="ExternalInput")
    w2_d = nc.dram_tensor("w2t", [MT2, P, D3], BF16, kind="ExternalInput")
    w3_d = nc.dram_tensor("w3t", [P, KT3], BF16, kind="ExternalInput")
    gbe0_d = nc.dram_tensor("gbe0", [P, MT1, 2], F32, kind="ExternalInput")
    gbe1_d = nc.dram_tensor("gbe1", [P, MT2, 2], F32, kind="ExternalInput")
    gbe2_d = nc.dram_tensor("gbe2", [P, MT3, 2], F32, kind="ExternalInput")
    bout_d = nc.dram_tensor("bout3", [1, 1], F32, kind="ExternalInput")
    out_d = nc.dram_tensor("out", [1, bc], F32, kind="ExternalOutput")

    with tile.TileContext(nc) as tc:
        with (
            tc.tile_pool(name="sb", bufs=1) as sb,
            tc.tile_pool(name="wpool", bufs=4) as wpool,
            tc.tile_pool(name="pspool", bufs=8, space="PSUM") as pspool,
            tc.tile_pool(name="small", bufs=1) as small,
            tc.tile_pool(name="dram", bufs=1, space="DRAM") as dram,
        ):
            # ---- persistent activations ----
            X = sb.tile([P, 2, bc], BF16, tag="X", name="X")

            nc.sync.dma_start(X[:], xt_d.ap().rearrange("t p b -> p t b"))
            # PII on partitions 0-63, PJJ on 64-127: the two K=64 selection
            # matmuls then run CONCURRENTLY as 64x128 row tiles (T0/T8).
            pp_sb = small.tile([P, NPAIR_PAD], BF16, tag="pp_sb", name="pp_sb")
            nc.sync.dma_start(pp_sb[0:FIELD, :], pii_d.ap())
            nc.sync.dma_start(pp_sb[FIELD:P, :], pjj_d.ap())
            # duplicate of x features 0-63 on partitions 64-127 (T8's rhs
            # must stream from SBUF partitions 64-127)
            xx_sb = small.tile([P, bc], BF16, tag="xx_sb", name="xx_sb")
            nc.sync.dma_start(xx_sb[FIELD:P, :], xt_d.ap()[0, 0:FIELD, :])

            gbe_sb = {}
            for li, (gbe_d, mt_n) in enumerate(
                    [(gbe0_d, MT1), (gbe1_d, MT2), (gbe2_d, MT3)]):
                g = small.tile([P, mt_n, 2], F32, tag=f"gbe{li}", name=f"gbe{li}")
                nc.sync.dma_start(g[:], gbe_d.ap())
                gbe_sb[li] = g
            w3_sb = small.tile([P, KT3], BF16, tag="w3_sb", name="w3_sb")
            nc.sync.dma_start(w3_sb[:], w3_d.ap())
            bout_sb = small.tile([1, 1], F32, tag="bout_sb", name="bout_sb")
            nc.sync.dma_start(bout_sb[:], bout_d.ap())
            eps_sb = small.tile([P, 1], F32, tag="eps_sb", name="eps_sb")
            nc.vector.memset(eps_sb[:], EPS)

            for _rep_ in range(net_reps):
                emit_forward(nc, tc, sb, wpool, pspool, small, dram,
                             bc, nch, mm_reps, X, pp_sb, xx_sb, gbe_sb,
                             w3_sb, bout_sb, eps_sb,
                             w0_d, w1_d, w2_d, out_d)

    nc.compile()
    return nc


def emit_forward(nc, tc, sb, wpool, pspool, small, dram, bc, nch, mm_reps,
                 X, pp_sb, xx_sb, gbe_sb, w3_sb, bout_sb, eps_sb,
                 w0_d, w1_d, w2_d, out_d):
    if True:
        if True:
            # "big" tag cycles: Q(qi) -> A1(act1) -> A2(act2) -> A3(act3)
            Q = sb.tile([P, 16, bc], BF16, tag="big", name="Q")
            # "h" tag cycles: H0 -> H1 -> H2 (pre-BN outputs)
            H0 = sb.tile([P, 16, bc], BF16, tag="h", name="H0")

            # ---- build qi tiles: Q[:, q, c] = (PII@x * (1/255)) * (PJJ@x) ----
            # DVE two-tensor ops allow at most one PSUM operand, so the
            # PJJ product bounces through SBUF (values are bf16-exact).
            for q in range(16):
                for c in range(nch):
                    a_ps = pspool.tile([P, NCHUNK], F32, tag="ps", name="aps")
                    b_ps = pspool.tile([P, NCHUNK], F32, tag="ps", name="bps")
                    nc.tensor.matmul(a_ps[:], pp_sb[0:FIELD, ts(q, P)],
                                     X[0:FIELD, 0, ts(c, NCHUNK)],
                                     start=True, stop=True,
                                     tile_position=(0, 0))
                    nc.tensor.matmul(b_ps[:], pp_sb[FIELD:P, ts(q, P)],
                                     xx_sb[FIELD:P, ts(c, NCHUNK)],
                                     start=True, stop=True,
                                     tile_position=(64, 0))
                    b_sb = wpool.tile([P, NCHUNK], BF16, tag="qtmp",
                                      name="b_sb", bufs=3)
                    nc.scalar.copy(b_sb[:], b_ps[:])
                    nc.vector.scalar_tensor_tensor(
                        out=Q[:, q, ts(c, NCHUNK)],
                        in0=a_ps[:], scalar=1.0 / 255.0, in1=b_sb[:],
                        op0=ALU.mult, op1=ALU.mult)

            def l0_rhs(kt, c):
                if kt < 2:
                    return X[:, kt, ts(c, NCHUNK)]
                return Q[:, kt - 2, ts(c, NCHUNK)]

            def dense_bn_relu(li, kt_n, mt_n, w_d, rhs_fn, h_buf):
                """h = act @ W.T ; sync-BN ; relu(s*h + t) IN-PLACE on h_buf.

                Sync-BN is split into two per-half AllReduces so the first
                one hides under the second half's matmuls, and the relu of
                each half runs on ACT (even m-tiles) + DVE (odd) in parallel,
                overlapping the tail AR / next layer's first k-loops.
                """
                stats6 = small.tile([P, mt_n, nch, 6], F32, tag=f"st6_{li}", name=f"st6_{li}")
                mv = small.tile([P, mt_n, 2], F32, tag=f"mv_{li}", name=f"mv_{li}")
                # asymmetric split: the big first AR hides under the last
                # quarter's matmuls; the small tail AR has more relu'd
                # k-tiles of PE cover in the next layer.
                s_pt = (3 * mt_n) // 4
                for half, (h0, h1) in enumerate([(0, s_pt), (s_pt, mt_n)]):
                    HALF = h1 - h0
                    for mt in range(h0, h1):
                        for _rep in range(mm_reps):
                            w_sb = wpool.tile([P, kt_n, P], BF16, tag="w",
                                              name="w_sb")
                            nc.sync.dma_start(
                                w_sb[:],
                                w_d.ap()[:, :, ts(mt, P)]
                                .rearrange("k p m -> p k m"))
                            ps = [pspool.tile([P, NCHUNK], F32, tag="ps",
                                              name="mps")
                                  for _ in range(nch)]
                            for kt in range(kt_n):
                                for c in range(nch):
                                    nc.tensor.matmul(ps[c][:], w_sb[:, kt, :],
                                                     rhs_fn(kt, c),
                                                     start=(kt == 0),
                                                     stop=(kt == kt_n - 1))
                            for c in range(nch):
                                nc.scalar.copy(h_buf[:, mt, ts(c, NCHUNK)],
                                               ps[c][:])
                        for c in range(nch):
                            nc.vector.bn_stats(stats6[:, mt, c, :],
                                               h_buf[:, mt, ts(c, NCHUNK)])
                        nc.vector.bn_aggr(mv[:, mt, :], stats6[:, mt, :, :])

                    # pack (mean/8, E[x^2]/8) for this half and AllReduce
                    hs = f"{li}_{half}"
                    mvh = mv[:, h0:h0 + HALF, :]
                    arp = small.tile([P, HALF, 2], F32, tag=f"arp_{hs}", name=f"arp_{hs}")
                    nc.vector.tensor_scalar_mul(arp[:, :, 0], mvh[:, :, 0],
                                                1.0 / N_CORES)
                    e2 = small.tile([P, HALF], F32, tag=f"e2_{hs}", name=f"e2_{hs}")
                    nc.vector.tensor_mul(e2[:], mvh[:, :, 0], mvh[:, :, 0])
                    nc.vector.tensor_add(e2[:], e2[:], mvh[:, :, 1])
                    nc.vector.tensor_scalar_mul(arp[:, :, 1], e2[:], 1.0 / N_CORES)
                    arin = dram.tile([P, HALF, 2], F32, tag=f"arin_{hs}", name=f"arin_{hs}")
                    arout = dram.tile([P, HALF, 2], F32, tag=f"arout_{hs}", name=f"arout_{hs}")
                    nc.sync.dma_start(arin[:], arp[:])
                    nc.gpsimd.collective_compute(
                        "AllReduce", ALU.add,
                        replica_groups=[list(range(N_CORES))],
                        ins=[arin.opt()], outs=[arout.opt()])
                    gl = small.tile([P, HALF, 2], F32, tag=f"gl_{hs}", name=f"gl_{hs}")
                    nc.sync.dma_start(gl[:], arout[:])

                    # s = g / sqrt(var+eps) ; t = be - mean*s
                    var = small.tile([P, HALF], F32, tag=f"var_{hs}", name=f"var_{hs}")
                    nc.vector.tensor_mul(var[:], gl[:, :, 0], gl[:, :, 0])
                    nc.vector.tensor_sub(var[:], gl[:, :, 1], var[:])
                    sd = small.tile([P, HALF], F32, tag=f"sd_{hs}", name=f"sd_{hs}")
                    nc.scalar.activation(sd[:], var[:], AF.Sqrt, bias=eps_sb[:])
                    s_t = small.tile([P, HALF], F32, tag=f"s_{hs}", name=f"s_{hs}")
                    nc.vector.reciprocal(s_t[:], sd[:])
                    nc.vector.tensor_mul(s_t[:], s_t[:],
                                         gbe_sb[li][:, h0:h0 + HALF, 0])
                    t_t = small.tile([P, HALF], F32, tag=f"t_{hs}", name=f"t_{hs}")
                    nc.vector.tensor_mul(t_t[:], gl[:, :, 0], s_t[:])
                    nc.vector.tensor_sub(t_t[:], gbe_sb[li][:, h0:h0 + HALF, 1],
                                         t_t[:])

                    for j in range(HALF):
                        mt = h0 + j
                        s_ap = s_t[:, j:j + 1]
                        t_ap = t_t[:, j:j + 1]
                        if j % 2 == 0:
                            nc.scalar.activation(h_buf[:, mt, :],
                                                 h_buf[:, mt, :], AF.Relu,
                                                 bias=t_ap, scale=s_ap)
                        else:
                            nc.vector.tensor_scalar(
                                out=h_buf[:, mt, :], in0=h_buf[:, mt, :],
                                scalar1=s_ap, scalar2=t_ap,
                                op0=ALU.mult, op1=ALU.add)
                            nc.vector.tensor_scalar_max(
                                h_buf[:, mt, :], h_buf[:, mt, :], 0.0)

            # ---- layer 0: reads X+Q, h0 -> H0 (relu'd in place) ----
            dense_bn_relu(0, KT0, MT1, w0_d, l0_rhs, H0)

            # ---- layer 1: reads H0, h1 -> H1 (reuses Q's memory) ----
            H1 = sb.tile([P, 16, bc], BF16, tag="big", name="H1")
            dense_bn_relu(1, MT1, MT2, w1_d,
                          lambda kt, c: H0[:, kt, ts(c, NCHUNK)], H1)

            # ---- layer 2: reads H1, h2 -> H2 (reuses H0's memory) ----
            H2 = sb.tile([P, 16, bc], BF16, tag="h", name="H2")
            dense_bn_relu(2, MT2, MT3, w2_d,
                          lambda kt, c: H1[:, kt, ts(c, NCHUNK)], H2)

            # ---- output layer: out[1, bc] = act3 @ Wout.T + bout ----
            out_sb = small.tile([1, bc], F32, tag="out_sb", name="out_sb")
            for c in range(nch):
                ps3 = pspool.tile([P, NCHUNK], F32, tag="ps", name="ps3")
                for kt in range(KT3):
                    nc.tensor.matmul(ps3[0:1, :], w3_sb[:, kt:kt + 1],
                                     H2[:, kt, ts(c, NCHUNK)],
                                     start=(kt == 0), stop=(kt == KT3 - 1))
                nc.scalar.activation(out_sb[:, ts(c, NCHUNK)], ps3[0:1, :],
                                     AF.Identity, bias=bout_sb[:])
            nc.sync.dma_start(out_d.ap(), out_sb[:])


# ---------------------------------------------------------------------------
# host-side prep + execution
# ---------------------------------------------------------------------------

_NC_CACHE = {}


def _get_nc(bc=BC, mm_reps=1, net_reps=1):
    key = (bc, mm_reps, net_reps)
    if key not in _NC_CACHE:
        _NC_CACHE[key] = build_nc(bc, mm_reps, net_reps)
    return _NC_CACHE[key]


def _bf16(a):
    return np.ascontiguousarray(a).astype(ml_dtypes.bfloat16)


def prep_in_maps(inputs, bc=BC, n_cores=N_CORES):
    xv = np.asarray(inputs["xv"], dtype=np.float32)
    W0 = np.asarray(inputs["W0"], dtype=np.float32)
    W1 = np.asarray(inputs["W1"], dtype=np.float32)
    W2 = np.asarray(inputs["W2"], dtype=np.float32)
    Wout = np.asarray(inputs["Wout"], dtype=np.float32)
    bout = np.asarray(inputs["bout"], dtype=np.float32)

    pii = np.zeros((FIELD, NPAIR_PAD), np.float32)
    pjj = np.zeros((FIELD, NPAIR_PAD), np.float32)
    pii[_II, np.arange(NPAIR)] = 1.0
    pjj[_JJ, np.arange(NPAIR)] = 1.0

    w0t = np.vstack([W0.T, np.zeros((KT0 * P - (F + NPAIR), D1), np.float32)])
    shared = {
        "pii": _bf16(pii),
        "pjj": _bf16(pjj),
        "w0t": _bf16(w0t.reshape(KT0, P, D1)),
        "w1t": _bf16(W1.T.reshape(MT1, P, D2)),
        "w2t": _bf16(W2.T.reshape(MT2, P, D3)),
        "w3t": _bf16(Wout.reshape(KT3, P).T),
        "bout3": bout.reshape(1, 1).astype(np.float32),
    }
    for li, (g, be, mt_n) in enumerate([
            (inputs["g0"], inputs["be0"], MT1),
            (inputs["g1"], inputs["be1"], MT2),
            (inputs["g2"], inputs["be2"], MT3)]):
        g = np.asarray(g, np.float32).reshape(mt_n, P).T
        be = np.asarray(be, np.float32).reshape(mt_n, P).T
        shared[f"gbe{li}"] = np.ascontiguousarray(
            np.stack([g, be], axis=-1), dtype=np.float32)

    in_maps = []
    for c in range(n_cores):
        xs = xv[c * bc:(c + 1) * bc, :]                     # [bc, F]
        xt = _bf16(xs.T.reshape(2, P, bc))
        m = dict(shared)
        m["xt"] = xt
        in_maps.append(m)
    return in_maps


def kernel(**inputs):
    nc = _get_nc(BC)
    in_maps = prep_in_maps(inputs)
    res = run_bass_kernel_spmd(nc, in_maps, core_ids=list(range(N_CORES)))
    out = np.concatenate(
        [res.results[c]["out"].reshape(BC) for c in range(N_CORES)])
    return out.reshape(B, 1).astype(np.float32)



# revision 23
# speedup vs baseline: 1.0719x; 1.0719x over previous
"""DeepQI dense MLP on 8 Trainium2 NeuronCores.

Strategy (data-parallel, feature-major):
  - Shard batch B=16384 across 8 cores (2048 rows each); weights replicated.
  - On-chip layout is feature-major: activations live as [128, KT, B_c]
    SBUF tiles (feature on partitions, batch on free dim), so every layer is
    a matmul with K on partitions, zero transposes between layers, and
    BatchNorm stats are free-dim reductions.
  - Pairwise features qi.T = (x_i * x_j)/255 are built on-chip: A = PII @ xT,
    Bm = PJJ @ xT via 0/1 selection matmuls (K=64), then one DVE
    scalar_tensor_tensor pass (A * (1/255)) * Bm -> bf16.
  - BatchNorm is sync-BN: per-core bn_stats/bn_aggr (DVE), then the per-core
    (mean/8, E[x^2]/8) pairs are exchanged with a small AllGather + local
    tree-sum on DVE (an AllGather has ~half the latency of an AllReduce in
    the collective stack), then scale/shift+relu fused per m-tile.
    Linear biases b0/b1/b2 cancel in BN and are dropped entirely.
  - Each layer's stats sync is split into a large leading group and a 2-tile
    tail group, so the leading AllGather hides under the tail's matmuls and
    the tail AllGather hides under the next layer's first k-loops.  The next
    layer's first TWO m-tiles are emitted with a split k-loop (ready k-tiles
    for both m-tiles first, AR-gated tail k-tiles after), doubling the PE
    cover for the tail AllGather to ~24us.
  - All matmuls in bf16 (fp32 PSUM accumulate): measured end-to-end rel err
    ~7e-3 vs the fp32 reference.
"""

import numpy as np
import ml_dtypes

import concourse.bass as bass
import concourse.mybir as mybir
import concourse.tile as tile
from concourse import bacc
from concourse.bass import ts
from concourse.bass_utils import run_bass_kernel_spmd

N_CORES = 8
P = 128
B = 16384
BC = B // N_CORES  # 2048 batch rows per core
F = 256            # xv feature dim
FIELD = 64
NPAIR = 2016
NPAIR_PAD = 2048   # pad pairs to 16 full k-tiles
D1, D2, D3 = 2048, 2048, 1024
NCHUNK = 512       # matmul moving free dim (one PSUM bank)
EPS = 1e-5
TAIL = 2           # m-tiles in each layer's trailing stats group

BF16 = mybir.dt.bfloat16
F32 = mybir.dt.float32
AF = mybir.ActivationFunctionType
ALU = mybir.AluOpType

_II, _JJ = np.triu_indices(FIELD, k=1)

KT0 = (F + NPAIR_PAD) // P          # 18
MT1, MT2, MT3 = D1 // P, D2 // P, D3 // P   # 16, 16, 8
KT3 = D3 // P                        # 8


def build_nc(bc=BC, net_reps=1):
    # net_reps > 1 redundantly re-runs the ENTIRE forward pass (identical
    # results) — used only to amplify device time above the axon relay's
    # wall-clock noise floor for timing.
    assert bc % NCHUNK == 0
    nch = bc // NCHUNK

    nc = bacc.Bacc("TRN2", target_bir_lowering=False, debug=False,
                   num_devices=N_CORES)

    xt_d = nc.dram_tensor("xt", [2, P, bc], BF16, kind="ExternalInput")
    pii_d = nc.dram_tensor("pii", [FIELD, NPAIR_PAD], BF16, kind="ExternalInput")
    pjj_d = nc.dram_tensor("pjj", [FIELD, NPAIR_PAD], BF16, kind="ExternalInput")
    w0_d = nc.dram_tensor("w0t", [KT0, P, D1], BF16, kind="ExternalInput")
    w1_d = nc.dram_tensor("w1t", [MT1, P, D2], BF16, kind="ExternalInput")
    w2_d = nc.dram_tensor("w2t", [MT2, P, D3], BF16, kind="ExternalInput")
    w3_d = nc.dram_tensor("w3t", [P, KT3], BF16, kind="ExternalInput")
    gbe0_d = nc.dram_tensor("gbe0", [P, MT1, 2], F32, kind="ExternalInput")
    gbe1_d = nc.dram_tensor("gbe1", [P, MT2, 2], F32, kind="ExternalInput")
    gbe2_d = nc.dram_tensor("gbe2", [P, MT3, 2], F32, kind="ExternalInput")
    bout_d = nc.dram_tensor("bout3", [1, 1], F32, kind="ExternalInput")
    out_d = nc.dram_tensor("out", [1, bc], F32, kind="ExternalOutput")

    with tile.TileContext(nc) as tc:
        with (
            tc.tile_pool(name="sb", bufs=1) as sb,
            tc.tile_pool(name="wpool", bufs=4) as wpool,
            tc.tile_pool(name="pspool", bufs=8, space="PSUM") as pspool,
            tc.tile_pool(name="small", bufs=1) as small,
            tc.tile_pool(name="dram", bufs=1, space="DRAM") as dram,
        ):
            # ---- persistent inputs ----
            X = sb.tile([P, 2, bc], BF16, tag="X", name="X")
            nc.sync.dma_start(X[:], xt_d.ap().rearrange("t p b -> p t b"))
            # PII on partitions 0-63, PJJ on 64-127: the two K=64 selection
            # matmuls then run CONCURRENTLY as 64x128 row tiles (T0/T8).
            pp_sb = small.tile([P, NPAIR_PAD], BF16, tag="pp_sb", name="pp_sb")
            nc.sync.dma_start(pp_sb[0:FIELD, :], pii_d.ap())
            nc.sync.dma_start(pp_sb[FIELD:P, :], pjj_d.ap())
            # duplicate of x features 0-63 on partitions 64-127 (T8's rhs
            # must stream from SBUF partitions 64-127)
            xx_sb = small.tile([P, bc], BF16, tag="xx_sb", name="xx_sb")
            nc.sync.dma_start(xx_sb[FIELD:P, :], xt_d.ap()[0, 0:FIELD, :])

            gbe_sb = {}
            for li, (gbe_d, mt_n) in enumerate(
                    [(gbe0_d, MT1), (gbe1_d, MT2), (gbe2_d, MT3)]):
                g = small.tile([P, mt_n, 2], F32, tag=f"gbe{li}", name=f"gbe{li}")
                nc.sync.dma_start(g[:], gbe_d.ap())
                gbe_sb[li] = g
            w3_sb = small.tile([P, KT3], BF16, tag="w3_sb", name="w3_sb")
            nc.sync.dma_start(w3_sb[:], w3_d.ap())
            bout_sb = small.tile([1, 1], F32, tag="bout_sb", name="bout_sb")
            nc.sync.dma_start(bout_sb[:], bout_d.ap())
            eps_sb = small.tile([P, 1], F32, tag="eps_sb", name="eps_sb")
            nc.vector.memset(eps_sb[:], EPS)

            for _rep_ in range(net_reps):
                emit_forward(nc, tc, sb, wpool, pspool, small, dram,
                             bc, nch, X, pp_sb, xx_sb, gbe_sb,
                             w3_sb, bout_sb, eps_sb,
                             w0_d, w1_d, w2_d, out_d)

    nc.compile()
    return nc


def emit_forward(nc, tc, sb, wpool, pspool, small, dram, bc, nch,
                 X, pp_sb, xx_sb, gbe_sb, w3_sb, bout_sb, eps_sb,
                 w0_d, w1_d, w2_d, out_d):
    # "big" tag cycles: Q(qi) -> A1(act1) -> A2(act2) -> A3(act3)
    Q = sb.tile([P, 16, bc], BF16, tag="big", name="Q")
    # "h" tag cycles: H0 -> H1 -> H2 (pre-BN outputs)
    H0 = sb.tile([P, 16, bc], BF16, tag="h", name="H0")

    def l0_rhs(kt, c):
        if kt < 2:
            return X[:, kt, ts(c, NCHUNK)]
        return Q[:, kt - 2, ts(c, NCHUNK)]

    # ---- build qi tiles: Q[:, q, c] = (PII@x * (1/255)) * (PJJ@x) ----
    # DVE two-tensor ops allow at most one PSUM operand, so the
    # PJJ product bounces through SBUF (values are bf16-exact).
    for q in range(16):
        for c in range(nch):
            a_ps = pspool.tile([P, NCHUNK], F32, tag="psc", name="aps",
                               bufs=4)
            b_ps = pspool.tile([P, NCHUNK], F32, tag="psh", name="bps",
                               bufs=4)
            nc.tensor.matmul(a_ps[:], pp_sb[0:FIELD, ts(q, P)],
                             X[0:FIELD, 0, ts(c, NCHUNK)],
                             start=True, stop=True,
                             tile_position=(0, 0))
            nc.tensor.matmul(b_ps[:], pp_sb[FIELD:P, ts(q, P)],
                             xx_sb[FIELD:P, ts(c, NCHUNK)],
                             start=True, stop=True,
                             tile_position=(64, 0))
            b_sb = wpool.tile([P, NCHUNK], BF16, tag="qtmp",
                              name="b_sb", bufs=3)
            nc.scalar.copy(b_sb[:], b_ps[:])
            nc.vector.scalar_tensor_tensor(
                out=Q[:, q, ts(c, NCHUNK)],
                in0=a_ps[:], scalar=1.0 / 255.0, in1=b_sb[:],
                op0=ALU.mult, op1=ALU.mult)

    def dense_bn_relu(li, kt_n, mt_n, w_d, rhs_fn, h_buf, prev_split=None,
                      pre_done=None):
        """h = act @ W.T ; sync-BN ; relu(s*h + t) IN-PLACE on h_buf.

        Stats sync is one AllGather per group (leading group of
        mt_n - TAIL tiles, trailing group of TAIL), summed locally on DVE.
        Emission is phase-split: BOTH groups' matmuls, local stats and
        AllGather issues go first, then both post-AG chains + relus — so
        the tail AllGather's prep ops don't queue behind the leading
        group's relu work on DVE/ACT.
        When prev_split is given, the first two m-tiles' k-loops are
        emitted split at prev_split so the PE has both tiles' ready
        k-work queued before the first AR-gated k-tile.
        """
        stats6 = small.tile([P, mt_n, nch, 6], F32, tag=f"st6_{li}",
                            name=f"st6_{li}")
        mv = small.tile([P, mt_n, 2], F32, tag=f"mv_{li}", name=f"mv_{li}")
        s_pt = mt_n - TAIL

        def start_mt(mt):
            w_sb = wpool.tile([P, kt_n, P], BF16, tag="w", name="w_sb")
            nc.sync.dma_start(
                w_sb[:],
                w_d.ap()[:, :, ts(mt, P)].rearrange("k p m -> p k m"))
            # alternate PSUM bank sets per m-tile for double buffering
            ps = [pspool.tile([P, NCHUNK], F32,
                              tag="psh" if mt % 2 == 0 else "psc",
                              name="mps", bufs=4)
                  for _ in range(nch)]
            return w_sb, ps

        def emit_k(w_sb, ps, k0, k1):
            for kt in range(k0, k1):
                for c in range(nch):
                    nc.tensor.matmul(ps[c][:], w_sb[:, kt, :],
                                     rhs_fn(kt, c),
                                     start=(kt == 0),
                                     stop=(kt == kt_n - 1))

        def finish_mt(mt, ps):
            # ACT copies PSUM->SBUF while DVE reads the same PSUM banks
            # for the stats, so neither engine waits on the other.
            for c in range(nch):
                nc.scalar.copy(h_buf[:, mt, ts(c, NCHUNK)], ps[c][:])
                nc.vector.bn_stats(stats6[:, mt, c, :], ps[c][:])
            nc.vector.bn_aggr(mv[:, mt, :], stats6[:, mt, :, :])

        # ---- phase A: matmuls, local stats, AllGather issues ----
        # the leading group is sub-split so the first AllGather launches
        # mid-layer and its relu'd tiles are ready well before the cover
        # matmuls consume them.
        mid = s_pt - 2
        groups = [(0, mid), (mid, s_pt), (s_pt, mt_n)]
        handles = []
        for gi, (h0, h1) in enumerate(groups):
            HALF = h1 - h0
            mts = list(range(h0, h1))
            if gi == 0 and prev_split is not None:
                wa, psa = start_mt(mts[0])
                wb, psb = start_mt(mts[1])
                emit_k(wa, psa, 0, prev_split)
                emit_k(wb, psb, 0, prev_split)
                emit_k(wa, psa, prev_split, kt_n)
                emit_k(wb, psb, prev_split, kt_n)
                finish_mt(mts[0], psa)
                finish_mt(mts[1], psb)
                mts = mts[2:]
            for mt in mts:
                if pre_done is not None and mt in pre_done:
                    finish_mt(mt, pre_done[mt])
                    continue
                w_sb, ps = start_mt(mt)
                emit_k(w_sb, ps, 0, kt_n)
                finish_mt(mt, ps)

            # pack (mean/8, E[x^2]/8) and issue the AllGather
            hs = f"{li}_{gi}"
            mvh = mv[:, h0:h0 + HALF, :]
            arp = small.tile([P, HALF, 2], F32, tag=f"arp_{hs}",
                             name=f"arp_{hs}")
            nc.vector.tensor_scalar_mul(arp[:, :, 0], mvh[:, :, 0],
                                        1.0 / N_CORES)
            m2 = small.tile([P, HALF], F32, tag=f"m2_{hs}", name=f"m2_{hs}")
            nc.vector.scalar_tensor_tensor(
                out=m2[:], in0=mvh[:, :, 0], scalar=1.0 / N_CORES,
                in1=mvh[:, :, 0], op0=ALU.mult, op1=ALU.mult)
            nc.vector.scalar_tensor_tensor(
                out=arp[:, :, 1], in0=mvh[:, :, 1], scalar=1.0 / N_CORES,
                in1=m2[:], op0=ALU.mult, op1=ALU.add)
            arin = dram.tile([P, HALF, 2], F32, tag=f"arin_{hs}",
                             name=f"arin_{hs}")
            arout = dram.tile([N_CORES, P, HALF, 2], F32, tag=f"arout_{hs}",
                              name=f"arout_{hs}")
            nc.sync.dma_start(arin[:], arp[:])
            nc.gpsimd.collective_compute(
                "AllGather", ALU.bypass,
                replica_groups=[list(range(N_CORES))],
                ins=[arin.opt()], outs=[arout.opt()])
            handles.append((gi, hs, h0, HALF, arout))

        # ---- phase B: post-AG tree-sum, scale/shift, relu ----
        def relu_act(dst, s_ap, t_ap):
            nc.scalar.activation(dst, dst, AF.Relu, bias=t_ap, scale=s_ap)

        def relu_dve(dst, s_ap, t_ap):
            nc.vector.tensor_scalar(out=dst, in0=dst,
                                    scalar1=s_ap, scalar2=t_ap,
                                    op0=ALU.mult, op1=ALU.add)
            nc.vector.tensor_scalar_max(dst, dst, 0.0)

        for gi, hs, h0, HALF, arout in handles:
            g8 = small.tile([P, N_CORES, HALF, 2], F32, tag=f"g8_{hs}",
                            name=f"g8_{hs}")
            nc.sync.dma_start(g8[:],
                              arout[:].rearrange("r p h t -> p r h t"))
            g4 = small.tile([P, 4, HALF, 2], F32, tag=f"g4_{hs}",
                            name=f"g4_{hs}")
            nc.vector.tensor_add(g4[:], g8[:, 0:4], g8[:, 4:8])
            g2 = small.tile([P, 2, HALF, 2], F32, tag=f"g2_{hs}",
                            name=f"g2_{hs}")
            nc.vector.tensor_add(g2[:], g4[:, 0:2], g4[:, 2:4])
            gl = small.tile([P, HALF, 2], F32, tag=f"gl_{hs}",
                            name=f"gl_{hs}")
            nc.vector.tensor_add(gl[:], g2[:, 0], g2[:, 1])

            # s = g * rsqrt(var+eps) ; t = be - mean*s
            var = small.tile([P, HALF], F32, tag=f"var_{hs}", name=f"var_{hs}")
            nc.vector.tensor_mul(var[:], gl[:, :, 0], gl[:, :, 0])
            nc.vector.tensor_sub(var[:], gl[:, :, 1], var[:])
            sd = small.tile([P, HALF], F32, tag=f"sd_{hs}", name=f"sd_{hs}")
            nc.scalar.activation(sd[:], var[:], AF.Sqrt, bias=eps_sb[:])
            s_t = small.tile([P, HALF], F32, tag=f"s_{hs}", name=f"s_{hs}")
            nc.vector.reciprocal(s_t[:], sd[:])
            nc.vector.tensor_mul(s_t[:], s_t[:],
                                 gbe_sb[li][:, h0:h0 + HALF, 0])
            t_t = small.tile([P, HALF], F32, tag=f"t_{hs}", name=f"t_{hs}")
            nc.vector.tensor_mul(t_t[:], gl[:, :, 0], s_t[:])
            nc.vector.tensor_sub(t_t[:], gbe_sb[li][:, h0:h0 + HALF, 1],
                                 t_t[:])

            if h0 < s_pt:
                for j in range(HALF):
                    mt = h0 + j
                    s_ap = s_t[:, j:j + 1]
                    t_ap = t_t[:, j:j + 1]
                    if mt % 2 == 0:
                        relu_act(h_buf[:, mt, :], s_ap, t_ap)
                    else:
                        relu_dve(h_buf[:, mt, :], s_ap, t_ap)
            else:
                # tail relu on the critical path: chunk it in c order,
                # alternating tiles across ACT/DVE so the next layer's
                # AR-gated k-tiles unblock chunk by chunk.
                for c in range(nch):
                    for j in range(HALF):
                        mt = h0 + j
                        s_ap = s_t[:, j:j + 1]
                        t_ap = t_t[:, j:j + 1]
                        dst = h_buf[:, mt, ts(c, NCHUNK)]
                        if (j + c) % 2 == 0:
                            relu_act(dst, s_ap, t_ap)
                        else:
                            relu_dve(dst, s_ap, t_ap)
        return kt_n

    # ---- layer 0: reads X+Q, h0 -> H0 (relu'd in place) ----
    dense_bn_relu(0, KT0, MT1, w0_d, l0_rhs, H0)

    # ---- layer 1: reads H0, h1 -> H1 (reuses Q's memory) ----
    H1 = sb.tile([P, 16, bc], BF16, tag="big", name="H1")
    dense_bn_relu(1, MT1, MT2, w1_d,
                  lambda kt, c: H0[:, kt, ts(c, NCHUNK)], H1,
                  prev_split=MT1 - TAIL)

    # ---- layer 2: reads H1, h2 -> H2 (reuses H0's memory) ----
    H2 = sb.tile([P, 16, bc], BF16, tag="h", name="H2")
    dense_bn_relu(2, MT2, MT3, w2_d,
                  lambda kt, c: H1[:, kt, ts(c, NCHUNK)], H2,
                  prev_split=MT2 - TAIL)

    # ---- output layer: out[1, bc] = act3 @ Wout.T + bout ----
    # k-outer with one PSUM accumulator per chunk, so the PE queues all
    # ready k-tiles across every chunk before the AR-gated tail k-tiles.
    out_sb = small.tile([1, bc], F32, tag="out_sb", name="out_sb")
    ps3 = [pspool.tile([P, NCHUNK], F32, tag="psh", name="ps3", bufs=4)
           for _ in range(nch)]
    for kt in range(KT3):
        for c in range(nch):
            nc.tensor.matmul(ps3[c][0:1, :], w3_sb[:, kt:kt + 1],
                             H2[:, kt, ts(c, NCHUNK)],
                             start=(kt == 0), stop=(kt == KT3 - 1))
    for c in range(nch):
        nc.scalar.activation(out_sb[:, ts(c, NCHUNK)], ps3[c][0:1, :],
                             AF.Identity, bias=bout_sb[:])
    nc.sync.dma_start(out_d.ap(), out_sb[:])


# ---------------------------------------------------------------------------
# host-side prep + execution
# ---------------------------------------------------------------------------

_NC_CACHE = {}


def _get_nc(bc=BC, net_reps=1):
    key = (bc, net_reps)
    if key not in _NC_CACHE:
        _NC_CACHE[key] = build_nc(bc, net_reps)
    return _NC_CACHE[key]


def _bf16(a):
    return np.ascontiguousarray(a).astype(ml_dtypes.bfloat16)


def prep_in_maps(inputs, bc=BC, n_cores=N_CORES):
    xv = np.asarray(inputs["xv"], dtype=np.float32)
    W0 = np.asarray(inputs["W0"], dtype=np.float32)
    W1 = np.asarray(inputs["W1"], dtype=np.float32)
    W2 = np.asarray(inputs["W2"], dtype=np.float32)
    Wout = np.asarray(inputs["Wout"], dtype=np.float32)
    bout = np.asarray(inputs["bout"], dtype=np.float32)

    pii = np.zeros((FIELD, NPAIR_PAD), np.float32)
    pjj = np.zeros((FIELD, NPAIR_PAD), np.float32)
    pii[_II, np.arange(NPAIR)] = 1.0
    pjj[_JJ, np.arange(NPAIR)] = 1.0

    w0t = np.vstack([W0.T, np.zeros((KT0 * P - (F + NPAIR), D1), np.float32)])
    shared = {
        "pii": _bf16(pii),
        "pjj": _bf16(pjj),
        "w0t": _bf16(w0t.reshape(KT0, P, D1)),
        "w1t": _bf16(W1.T.reshape(MT1, P, D2)),
        "w2t": _bf16(W2.T.reshape(MT2, P, D3)),
        "w3t": _bf16(Wout.reshape(KT3, P).T),
        "bout3": bout.reshape(1, 1).astype(np.float32),
    }
    for li, (g, be, mt_n) in enumerate([
            (inputs["g0"], inputs["be0"], MT1),
            (inputs["g1"], inputs["be1"], MT2),
            (inputs["g2"], inputs["be2"], MT3)]):
        g = np.asarray(g, np.float32).reshape(mt_n, P).T
        be = np.asarray(be, np.float32).reshape(mt_n, P).T
        shared[f"gbe{li}"] = np.ascontiguousarray(
            np.stack([g, be], axis=-1), dtype=np.float32)

    in_maps = []
    for c in range(n_cores):
        xs = xv[c * bc:(c + 1) * bc, :]                     # [bc, F]
        xt = _bf16(xs.T.reshape(2, P, bc))
        m = dict(shared)
        m["xt"] = xt
        in_maps.append(m)
    return in_maps


def kernel(**inputs):
    nc = _get_nc(BC)
    in_maps = prep_in_maps(inputs)
    res = run_bass_kernel_spmd(nc, in_maps, core_ids=list(range(N_CORES)))
    out = np.concatenate(
        [res.results[c]["out"].reshape(BC) for c in range(N_CORES)])
    return out.reshape(B, 1).astype(np.float32)
